# revision 67
# baseline (speedup 1.0000x reference)
"""ClusterAttention Trainium2 kernel — 3-phase design.

Phase P (proj): token-order qkv projection, shared across heads.
  Each core handles (b, token-half): qkv[1152, 4096] = W^T-chunks.T @ featT,
  o-major, bf16 in/out, fp32 psum. Host pre-scales Wq rows by softmax scale.
Host gather: per (b,h) row, gather q/k/v columns into cluster order, build
  augmented q/k (20 rows: 16 qk dims + bias/pos-bias fold rows) and t-major
  v with a ones column (softmax denominator via matmul).
Phase A (attention): per core 6 rows; per cluster S'=k_aug.T@q_aug ->
  exp on ACT (psum->sbuf bf16) -> transposed AV: O[c,i] = sum_j v_t[j,c]E[j,i]
  with 256-wide moving dim; row 64 of O = denominator. Out o-major, bf16,
  unnormalized (host divides by denominator).
Host scatter: normalize, scatter to token order, build feat2T per (b, half).
Phase B (proj): outT[384, 4096] = w_proj chunks.T @ feat2T + bias.

All matmuls bf16 (1 cycle/col vs 4 for fp32); psum->sbuf copies spread over
DVE and ACT; big out DMAs ride the idle Pool SWDGE queue; all DMAs keep
>=512B contiguous runs (below that the model halves DMA bandwidth).
"""
import os
import numpy as np
import ml_dtypes

import concourse.bacc as bacc
import concourse.tile as tile
from concourse import mybir
from concourse.bass_utils import run_bass_kernel_spmd

B, N, C, H, D, K, M = 4, 8192, 384, 12, 2, 32, 256
CH = C // H // 2            # 16
BH = B * H                  # 48
R = BH // 8                 # 6 rows (heads) per core in phase A
SCALE = float((C // H) ** -0.5)
TPB = N * B // 8            # 4096 tokens per core in phases P and B

F32 = mybir.dt.float32
BF16 = mybir.dt.bfloat16
NPBF = ml_dtypes.bfloat16
EXP = mybir.ActivationFunctionType.Exp
# copy-engine set: d=DVE, s=ACT(scalar), g=Pool(gpsimd, breaks device lowering);
# round-robined in order
KCOPY = os.environ.get("KCOPY", "sd")
BOUT = os.environ.get("BOUT", "bf16")  # phase-B output dtype


def _copy_psum(nc, dst, src, i, eng=None):
    eng = eng or KCOPY
    c = eng[i % len(eng)]
    if c == "d":
        nc.vector.tensor_copy(dst, src)
    elif c == "s":
        nc.scalar.copy(dst, src)
    else:
        nc.gpsimd.tensor_copy(dst, src)


def build_phase_p():
    nc = bacc.Bacc(None, target_bir_lowering=False)
    ft = nc.dram_tensor("ft", [3 * 128, TPB], BF16, kind="ExternalInput")
    wt = nc.dram_tensor("wt", [3 * 128, 9 * 128], BF16, kind="ExternalInput")
    qkv = nc.dram_tensor("qkv", [9 * 128, TPB], BF16, kind="ExternalOutput")
    with tile.TileContext(nc) as tc:
        with (
            tc.tile_pool(name="sb", bufs=1) as pool,
            tc.tile_pool(name="sb_o", bufs=1) as p_o,
            tc.tile_pool(name="ps", bufs=4, space="PSUM") as ps,
        ):
            ft_sb = pool.tile([128, 3 * TPB], BF16, tag="ft")
            w_sb = pool.tile([128, 3 * 1152], BF16, tag="w")
            warm = pool.tile([20, 256], BF16, tag="warm")
            nc.vector.memset(warm[:, :], 0.0)
            wps = ps.tile([128, 512], F32, tag="p")
            for i in range(10):
                nc.tensor.matmul(wps[:, 0:256], warm[:, 0:128], warm[:, 0:256],
                                 start=True, stop=True)
            # fused DMAs (3 cc chunks in one 3D access pattern each)
            ftr = ft.rearrange("(c p) t -> p c t", p=128)
            ftv = ft_sb.rearrange("p (c t) -> p c t", t=TPB)
            wtr = wt.rearrange("(c p) j -> p c j", p=128)
            wtv = w_sb.rearrange("p (c j) -> p c j", j=1152)
            # oc=0 weight slice first so the first matmul isn't gated on all of w
            nc.sync.dma_start(wtv[:, :, 0:256], wtr[:, :, 0:256])
            nc.sync.dma_start(ftv[:, :, 0:512], ftr[:, :, 0:512])
            nc.sync.dma_start(wtv[:, :, 256:640], wtr[:, :, 256:640])
            nc.sync.dma_start(wtv[:, :, 640:], wtr[:, :, 640:])
            for q0 in range(512, TPB, 512):
                nc.sync.dma_start(ftv[:, :, q0:q0 + 512], ftr[:, :, q0:q0 + 512])
            o_all = pool.tile([128, 9 * TPB], BF16, tag="o_all")
            o_v = o_all.rearrange("p (c t) -> p c t", t=TPB)
            qkv_v = qkv.rearrange("(c p) t -> p c t", p=128)
            # token tiles taper at the end so the final drain DMA is tiny;
            # fused 3D out DMAs (all 9 oc at once) keep the HWDGE count low
            widths = [512] * 7 + [256, 256]
            drains = [(0, 1024), (1024, 1024), (2048, 1024), (3072, 512),
                      (3584, 256), (3840, 256)]
            t0 = 0
            di = 0
            for tt, w in enumerate(widths):
                for oc in range(9):
                    p = ps.tile([128, 512], F32, tag="p")
                    for cc in range(3):
                        nc.tensor.matmul(
                            p[:, 0:w],
                            w_sb[:, cc * 1152 + oc * 128: cc * 1152 + (oc + 1) * 128],
                            ft_sb[:, cc * TPB + t0: cc * TPB + t0 + w],
                            start=(cc == 0), stop=(cc == 2))
                    _copy_psum(nc, o_v[:, oc, t0:t0 + w], p[:, 0:w], oc)
                t0 += w
                while di < len(drains) and drains[di][0] + drains[di][1] <= t0:
                    d0, dw = drains[di]
                    di += 1
                    nc.sync.dma_start(qkv_v[:, :, d0:d0 + dw],
                                      o_v[:, :, d0:d0 + dw])
    nc.compile()
    return nc


def build_phase_a():
    nc = bacc.Bacc(None, target_bir_lowering=False)
    qk = nc.dram_tensor("qk", [R * 40, N], BF16, kind="ExternalInput")
    vt = nc.dram_tensor("vt", [R * 128, 64 * 65], BF16, kind="ExternalInput")
    og = nc.dram_tensor("og", [R * 65, N], BF16, kind="ExternalOutput")
    with tile.TileContext(nc) as tc:
        with (
            tc.tile_pool(name="row", bufs=2) as p_row,
            tc.tile_pool(name="e", bufs=int(os.environ.get("KEB", "5"))) as p_e,
            tc.tile_pool(name="ps_s", bufs=(3 if os.environ.get("KGRP", "3f") == "2" else 2),
                         space="PSUM") as ps_s,
            tc.tile_pool(name="ps_o", bufs=2, space="PSUM") as ps_o,
        ):
            NP_ = K // 2  # cluster pairs per row
            rows = {}

            # warm the ACT exp table at t=0 so the 1.3us load hides under DMA,
            # and run dummy matmuls so the PE p-state ramps while DMAs fly
            scratch = p_e.tile([1, 8], F32, tag="warm")
            nc.vector.memset(scratch[:, :], 0.0)
            nc.scalar.activation(scratch[:, 4:8], scratch[:, 0:4], EXP)
            wsb = p_e.tile([20, 256], BF16, tag="wsb")
            nc.vector.memset(wsb[:, :], 0.0)
            wps = ps_o.tile([128, 512], F32, tag="po")
            for i in range(14):
                nc.tensor.matmul(wps[:, 0:256], wsb[:, 0:128], wsb[:, 0:256],
                                 start=True, stop=True)

            def load_row(r):
                q_sb = p_row.tile([20, N], BF16, tag="q")
                k_sb = p_row.tile([20, N], BF16, tag="k")
                v_sb = p_row.tile([128, 64 * 65], BF16, tag="v")
                o_sb = p_row.tile([65, N], BF16, tag="o")
                # chunked so the first clusters' operands land early
                for lo, hi in ((0, 256), (256, 2048), (2048, N)):
                    nc.sync.dma_start(q_sb[:, lo:hi], qk[r * 40: r * 40 + 20, lo:hi])
                    nc.sync.dma_start(k_sb[:, lo:hi],
                                      qk[r * 40 + 20: r * 40 + 40, lo:hi])
                nc.sync.dma_start(v_sb[:, 0: 8 * 65],
                                  vt[r * 128:(r + 1) * 128, 0: 8 * 65])
                nc.sync.dma_start(v_sb[:, 8 * 65:],
                                  vt[r * 128:(r + 1) * 128, 8 * 65:])
                rows[r] = (q_sb, k_sb, v_sb.rearrange("p (c w) -> p c w", w=65), o_sb)

            # Flat software pipeline over all clusters: QK+exp runs DEPTH_C
            # clusters ahead of AV+copy so ACT (the bottleneck) never starves.
            # S/E tiles batch 3 clusters ([128, 1536], 3 psum banks) to cut the
            # per-activation init overhead; AV works in 2-cluster units.
            # Input DMAs prefetch one row ahead on SP; out DMAs stream via the
            # idle Pool SWDGE queue, except each row's last two chunks which
            # ride SP/inline so the program tail is one small hop.
            DEPTH_C = int(os.environ.get("KDC", "9"))
            _g = os.environ.get("KGRP", "3f")
            GROUPS = {"3": [3] * 10 + [2], "3f": [2] + [3] * 10, "2": [2] * 16}[_g]
            sitems = []
            for r in range(R):
                c0 = 0
                for g in GROUPS:
                    sitems.append((r, c0, g))
                    c0 += g
            e_of = {}
            av_list = [(r, pp) for r in range(R) for pp in range(NP_)]
            av_ptr = 0
            issued = 0
            done = 0

            def do_av(eng="d"):
                nonlocal av_ptr, done
                qr, qp = av_list[av_ptr]
                av_ptr += 1
                done += 2
                _, _, v_view, o_sb = rows[qr]
                po = ps_o.tile([128, 512], F32, tag="po")
                for u in range(2):
                    et, off = e_of.pop((qr, qp * 2 + u))
                    for jc in range(2):
                        nc.tensor.matmul(
                            po[0:65, u * 256:(u + 1) * 256],
                            v_view[:, (qp * 2 + u) * 2 + jc, 0:65],
                            et[:, off + jc * 256: off + (jc + 1) * 256],
                            start=(jc == 0), stop=(jc == 1))
                _copy_psum(nc, o_sb[:, qp * 512:(qp + 1) * 512],
                           po[0:65, :], 0, eng=eng)
                # stream the row's output: big chunks via Pool SWDGE, the last
                # two pairs per-chunk on SP right behind their copies
                if qp == 7:
                    nc.gpsimd.dma_start(og[qr * 65:(qr + 1) * 65, 0:4096],
                                        o_sb[:, 0:4096])
                elif qp == 13:
                    nc.gpsimd.dma_start(og[qr * 65:(qr + 1) * 65, 4096:7168],
                                        o_sb[:, 4096:7168])
                elif qp >= 14:
                    nc.sync.dma_start(
                        og[qr * 65:(qr + 1) * 65, qp * 512:(qp + 1) * 512],
                        o_sb[:, qp * 512:(qp + 1) * 512])

            load_row(0)
            for r, c0, g in sitems:
                if c0 == 0 and r + 1 < R:
                    load_row(r + 1)
                # drain AV work first so PE has queued work while the next
                # S-group's psum recycles through the pending exp; taper the
                # lag over the last groups so the end-flush backlog is small.
                # At a row's first group the QK matmuls are the critical path
                # (fresh row data), so drain after issuing instead.
                lag = DEPTH_C + g
                left = len(sitems) - sitems.index((r, c0, g))
                _tn, _tm = (int(x) for x in os.environ.get("KTAP", "3,2").split(","))
                if left <= _tn:
                    lag = min(lag, _tm * left)
                if c0 > 0:
                    while av_ptr < len(av_list) and issued - done >= lag:
                        do_av()
                q_sb, k_sb, v_view, o_sb = rows[r]
                ps = ps_s.tile([128, 512 * g], F32, tag="s",
                               padded_shape=[128, 512 * max(GROUPS)])
                for i in range(g):
                    col = (c0 + i) * 256
                    nc.tensor.matmul(ps[:, i * 512: i * 512 + 256],
                                     k_sb[:, col:col + 128],
                                     q_sb[:, col:col + 256],
                                     start=True, stop=True)
                    nc.tensor.matmul(ps[:, i * 512 + 256: i * 512 + 512],
                                     k_sb[:, col + 128:col + 256],
                                     q_sb[:, col:col + 256],
                                     start=True, stop=True)
                e = p_e.tile([128, 512 * g], BF16, tag="e",
                             padded_shape=[128, 512 * max(GROUPS)])
                nc.scalar.activation(e[:, :], ps[:, :], EXP)
                for i in range(g):
                    e_of[(r, c0 + i)] = (e, i * 512)
                issued += g
                if c0 == 0:
                    while av_ptr < len(av_list) and issued - done >= lag:
                        do_av()
            flush_i = 0
            while av_ptr < len(av_list):
                do_av(eng="ds"[flush_i % 2])
                flush_i += 1
    nc.compile()
    return nc


def build_phase_b():
    dt_out = F32 if BOUT == "f32" else BF16
    nc = bacc.Bacc(None, target_bir_lowering=False)
    f2T = nc.dram_tensor("f2T", [6 * 128, TPB], BF16, kind="ExternalInput")
    wp2 = nc.dram_tensor("wp2", [6 * 128, 384], BF16, kind="ExternalInput")
    bias = nc.dram_tensor("bias", [3 * 128, 1], F32, kind="ExternalInput")
    outT = nc.dram_tensor("outT", [3 * 128, TPB], dt_out, kind="ExternalOutput")
    with tile.TileContext(nc) as tc:
        with (
            tc.tile_pool(name="sb", bufs=1) as pool,
            tc.tile_pool(name="sb_o", bufs=1) as p_o,
            tc.tile_pool(name="ps", bufs=4, space="PSUM") as ps,
        ):
            fsb = pool.tile([128, 6 * TPB], BF16, tag="fsb")
            wsb = pool.tile([128, 6 * 384], BF16, tag="wsb")
            bsb = pool.tile([128, 3], F32, tag="bsb")
            warm = pool.tile([20, 256], BF16, tag="warm")
            nc.vector.memset(warm[:, :], 0.0)
            wps = ps.tile([128, 512], F32, tag="p")
            for i in range(12):
                nc.tensor.matmul(wps[:, 0:256], warm[:, 0:128], warm[:, 0:256],
                                 start=True, stop=True)
            # fused DMAs (6 cc chunks in one 3D access pattern each)
            nc.sync.dma_start(
                bsb.rearrange("p (c j) -> p c j", j=1)[:, :, :],
                bias.rearrange("(c p) j -> p c j", p=128)[:, :, :])
            fr = f2T.rearrange("(c p) t -> p c t", p=128)
            fv = fsb.rearrange("p (c t) -> p c t", t=TPB)
            wr = wp2.rearrange("(c p) j -> p c j", p=128)
            wv = wsb.rearrange("p (c j) -> p c j", j=384)
            nc.sync.dma_start(wv[:, :, 0:256], wr[:, :, 0:256])
            nc.sync.dma_start(fv[:, :, 0:256], fr[:, :, 0:256])
            nc.sync.dma_start(wv[:, :, 256:], wr[:, :, 256:])
            nc.sync.dma_start(fv[:, :, 256:512], fr[:, :, 256:512])
            for q0 in range(512, TPB, 512):
                nc.sync.dma_start(fv[:, :, q0:q0 + 512], fr[:, :, q0:q0 + 512])
            o_all = pool.tile([128, 3 * TPB], dt_out, tag="o_all")
            o_v = o_all.rearrange("p (c t) -> p c t", t=TPB)
            out_v = outT.rearrange("(c p) t -> p c t", p=128)
            widths = [256, 256] + [512] * 6 + [256, 256]
            drains = [(0, 512), (512, 1024), (1536, 1024), (2560, 1024),
                      (3584, 256), (3840, 256)]
            t0 = 0
            di = 0
            for tt, w in enumerate(widths):
                for oc in range(3):
                    p = ps.tile([128, 512], F32, tag="p")
                    for cc in range(6):
                        nc.tensor.matmul(
                            p[:, 0:w],
                            wsb[:, cc * 384 + oc * 128: cc * 384 + (oc + 1) * 128],
                            fsb[:, cc * TPB + t0: cc * TPB + t0 + w],
                            start=(cc == 0), stop=(cc == 5))
                    nc.vector.tensor_scalar(o_v[:, oc, t0:t0 + w], p[:, 0:w],
                                            bsb[:, oc:oc + 1], None,
                                            mybir.AluOpType.add)
                t0 += w
                while di < len(drains) and drains[di][0] + drains[di][1] <= t0:
                    d0, dw = drains[di]
                    di += 1
                    nc.sync.dma_start(out_v[:, :, d0:d0 + dw],
                                      o_v[:, :, d0:d0 + dw])
    nc.compile()
    return nc


_CACHE = {}
PHASES = ("p", "a", "b")
_BUILDERS = {"p": build_phase_p, "a": build_phase_a, "b": build_phase_b}


def _get(name):
    if name not in _CACHE:
        _CACHE[name] = _BUILDERS[name]()
    return _CACHE[name]


def kernel(pos, feat, member_idx, w_qkv, b_qkv, w_pos, b_pos, w_proj, b_proj):
    import time
    pos = np.asarray(pos, np.float32)
    feat = np.asarray(feat, np.float32)
    mf = np.asarray(member_idx).astype(np.int64).reshape(BH, N)
    w_qkv = np.asarray(w_qkv, np.float32); b_qkv = np.asarray(b_qkv, np.float32)
    w_pos = np.asarray(w_pos, np.float32); b_pos = np.asarray(b_pos, np.float32)
    w_proj = np.asarray(w_proj, np.float32); b_proj = np.asarray(b_proj, np.float32)

    t0 = time.time()
    # ---- phase P host prep: featT per (b, half), prescaled w_qkv^T
    featT = np.ascontiguousarray(feat.transpose(0, 2, 1)).astype(NPBF)  # [B,C,N]
    w_s = w_qkv.copy()
    for h in range(H):
        w_s[h * 96: h * 96 + 16] *= SCALE          # fold softmax scale into Wq
    wt = np.ascontiguousarray(w_s.T).astype(NPBF)  # [384, 1152]
    in_p = []
    for c in range(8):
        b, half = divmod(c, 2)
        in_p.append({"ft": np.ascontiguousarray(featT[b][:, half * TPB:(half + 1) * TPB]),
                     "wt": wt})
    t1 = time.time()
    res_p = run_bass_kernel_spmd(_get("p"), in_p, core_ids=list(range(8)))
    t2 = time.time()

    # ---- host gather into cluster order + augmented rows
    qkv_all = [np.concatenate([res_p.results[2 * b]["qkv"],
                               res_p.results[2 * b + 1]["qkv"]], axis=1)
               for b in range(B)]                  # [1152, N] bf16 each
    pos_n = pos / pos.reshape(-1, D).max(0)
    b_of = np.repeat(np.arange(B), H)
    pos_g = np.take_along_axis(pos_n[b_of], mf[:, :, None], axis=1)   # [48,N,2]
    s_g = np.einsum('rnd,rd->rn', pos_g, np.tile(w_pos, (B, 1))).astype(np.float32)

    ones = np.ones((N,), NPBF)
    zeros = np.zeros((N,), NPBF)
    has_bias = bool(np.any(b_qkv))
    qk_host = np.empty((8, R * 40, N), NPBF)
    vt_host = np.empty((8, R * 128, 64 * 65), NPBF)
    for r in range(BH):
        b, h = divmod(r, H)
        core, rr = divmod(r, R)
        blk = qkv_all[b]
        idx = mf[r]
        qg = blk[h * 96: h * 96 + 16][:, idx]
        kg = blk[h * 96 + 16: h * 96 + 32][:, idx]
        vg = blk[h * 96 + 32: h * 96 + 96][:, idx]          # [64, N] bf16
        row2 = (-s_g[r]).astype(NPBF)
        row5 = (s_g[r] + b_pos[h]).astype(NPBF)
        rowqA, rowkB = zeros, zeros
        if has_bias:
            bq = b_qkv[h * 96: h * 96 + 16]
            bk = b_qkv[h * 96 + 16: h * 96 + 32]
            # qg already carries SCALE, so bk @ qg == scale*(bk . q_raw)
            rowqA = (bk @ qg.astype(np.float32)).astype(NPBF)
            row5 = (s_g[r] + b_pos[h] + SCALE * (bq @ kg.astype(np.float32))
                    + SCALE * float(bq @ bk)).astype(NPBF)
        qa = qk_host[core, rr * 40: rr * 40 + 20]
        qa[0:16] = qg; qa[16] = rowqA; qa[17] = ones; qa[18] = row2; qa[19] = ones
        ka = qk_host[core, rr * 40 + 20: rr * 40 + 40]
        ka[0:16] = kg; ka[16] = ones; ka[17] = row5; ka[18] = ones; ka[19] = rowkB
        vt = np.empty((N, 65), NPBF)
        vt[:, 0:64] = vg.T
        if has_bias:
            bv = np.concatenate([b_qkv[h * 96 + 32: h * 96 + 96]])
            vt[:, 0:64] = (vt[:, 0:64].astype(np.float32) + bv).astype(NPBF)
        vt[:, 64] = 1.0
        vt_host[core, rr * 128:(rr + 1) * 128] = (
            vt.reshape(64, 128, 65).transpose(1, 0, 2).reshape(128, 64 * 65))
    in_a = [{"qk": qk_host[c], "vt": vt_host[c]} for c in range(8)]
    t3 = time.time()
    res_a = run_bass_kernel_spmd(_get("a"), in_a, core_ids=list(range(8)))
    t4 = time.time()

    # ---- host: normalize, scatter to token order, build f2T
    f2T = np.empty((B, 2 * C, N), NPBF)
    for r in range(BH):
        b, h = divmod(r, H)
        core, rr = divmod(r, R)
        o = res_a.results[core]["og"][rr * 65:(rr + 1) * 65].astype(np.float32)
        on = o[0:64] / o[64:65]
        f2T[b][h * 64:(h + 1) * 64][:, mf[r]] = on.astype(NPBF)
    wp2 = np.ascontiguousarray(w_proj.T).astype(NPBF)       # [768, 384]
    b_eff = b_proj + w_proj @ np.concatenate(
        [b_qkv[h * 96 + 32: h * 96 + 96] for h in range(H)])
    in_b = []
    for c in range(8):
        b, half = divmod(c, 2)
        in_b.append({
            "f2T": np.ascontiguousarray(f2T[b][:, half * TPB:(half + 1) * TPB]),
            "wp2": wp2,
            "bias": b_eff.reshape(384, 1).astype(np.float32),
        })
    t5 = time.time()
    res_b = run_bass_kernel_spmd(_get("b"), in_b, core_ids=list(range(8)))
    t6 = time.time()

    out = np.empty((B, N, C), np.float32)
    for c in range(8):
        b, half = divmod(c, 2)
        out[b, half * TPB:(half + 1) * TPB, :] = \
            res_b.results[c]["outT"].astype(np.float32).T
    if os.environ.get("KTIME"):
        print(f"[kernel] prep1={t1-t0:.2f}s runP={t2-t1:.2f}s prep2={t3-t2:.2f}s "
              f"runA={t4-t3:.2f}s prep3={t5-t4:.2f}s runB={t6-t5:.2f}s")
    return out
